# revision 14
# baseline (speedup 1.0000x reference)
"""Fused single-head attention + residual + LayerNorm for Trainium2 (Bass/Tile).

Problem: B=4, S=4096, E=512 fp32.
  Q/K/V = x @ W^T + b ; S = QK^T/sqrt(E) ; mask keys ; softmax ; ctx = P@V ;
  out = LayerNorm(ctx + x) * gamma + beta

Sharding: 8 cores = 4 batches x 2 halves of the S=4096 rows. Each core
projects Q/K/V for its OWN 2048 rows only and runs attention + layernorm
for those rows; the two cores of a batch exchange their packed K^T/V with
per-chunk 2-core AllGathers pipelined behind the projections.

v2 strategy (vs the bf16 baseline):
  - All heavy matmuls in fp8e4 with MatmulPerfMode.DoubleRow: both
    operands are laid out [128, 2, free] so each matmul contracts 256
    rows at 0.5 PE cycles/row (2x+ the bf16 rate). The attention output
    ("context") is ~1.5% of the magnitude of the residual x, so fp8
    rounding in the attention path is strongly damped in the final
    output.
  - Masked keys are packed out at 128 granularity (not 512): nkt =
    ceil(max_unmasked/128) rounded even, so attention runs on 2*nkt
    k-tiles (20) instead of the 512-padded 24.
  - Scores are computed transposed S^T[k, q] (k on partitions) into one
    PSUM tile [128, 4qc, 512] per k-tile, so ONE ScalarE activation
    exp(s*scale + maskbias_k - 1) covers all 2048 q for that k-tile
    (the -1 shift keeps exp < 8 for fp8 headroom; softmax normalization
    cancels it). P is written directly in fp8 paired layout for ctx.
  - Row sums ride in the P@V matmul via a ones-column appended to V.
  - Single ctx accumulation over all 2*nkt k-tiles (no spill pass):
    the K/V pair exchange is hidden behind the projections + own-half
    scores. KV chunks are projected and gathered FIRST, before Q.
  - LayerNorm: h built by DVE scalar_tensor_tensor from PSUM, stats via
    bn_stats, sqrt on ScalarE (all sqrts happen after all exps -> one
    activation-table switch), normalize on GpSimd (SBUF-only engine).
"""

import sys

import numpy as np

sys.path.insert(0, "/opt/trn_rl_repo")

import concourse.bass as bass  # noqa: E402
import concourse.tile as tile  # noqa: E402
from concourse import bacc, mybir  # noqa: E402

E = 512
S = 4096  # keys per batch
SQ = 2048  # query rows per core
QC = SQ // 512  # 4   512-chunks along q
F32 = mybir.dt.float32
FP8 = mybir.dt.float8e4
SCALE = 1.0 / float(np.sqrt(E))
EPS = 1e-5
MASK_NEG = -10000.0
SHIFT = -1.0  # softmax-invariant score shift, keeps exp() in fp8 range
DR = mybir.MatmulPerfMode.DoubleRow


def build_nc(nkt, apply_gb):
    # nkt = own k-tiles of 128 packed (unmasked) keys, even
    assert nkt % 2 == 0
    S_OWN = nkt * 128
    KT = 2 * nkt  # total k-tiles (own + sibling)
    JP = nkt  # ctx pair-tiles of 256 keys
    JPO = nkt // 2  # own pair-tiles
    chunks = [(a, min(a + 512, S_OWN)) for a in range(0, S_OWN, 512)]

    nc = bacc.Bacc("TRN2", target_bir_lowering=False, debug=False)
    xq = nc.dram_tensor("xq", [SQ, E], F32, kind="ExternalInput")
    xkv = nc.dram_tensor("xkv", [S_OWN, E], F32, kind="ExternalInput")
    mbias = nc.dram_tensor("maskbias", [KT * 128], F32, kind="ExternalInput")
    WqT = nc.dram_tensor("WqT", [E, E], F32, kind="ExternalInput")
    WkT = nc.dram_tensor("WkT", [E, E], F32, kind="ExternalInput")
    WvT = nc.dram_tensor("WvT", [E, E], F32, kind="ExternalInput")
    bq = nc.dram_tensor("bq", [E], F32, kind="ExternalInput")
    bk = nc.dram_tensor("bk", [E], F32, kind="ExternalInput")
    bv = nc.dram_tensor("bv", [E], F32, kind="ExternalInput")
    gamma = nc.dram_tensor("gamma", [E], F32, kind="ExternalInput")
    beta = nc.dram_tensor("beta", [E], F32, kind="ExternalInput")
    ident_in = nc.dram_tensor("ident", [128, 128], F32, kind="ExternalInput")
    out = nc.dram_tensor("out", [SQ, E], F32, kind="ExternalOutput")

    AF = mybir.ActivationFunctionType
    OP = mybir.AluOpType

    def ap3(handle, offset, dims):
        a = handle[:]
        return bass.AP(tensor=a.tensor, offset=offset, ap=dims)

    with tile.TileContext(nc) as tc:
        with (
            tc.tile_pool(name="persist", bufs=1) as persist,
            tc.tile_pool(name="dram", bufs=1, space="DRAM") as dram,
        ):
            # ---------------- constants ----------------
            # dummy 128B AllGather issued first: starts the ~27us CC-core
            # init at t=0 so the real K/V gathers aren't serialized behind it
            warm_in = dram.tile([128], FP8, tag="warm_in")
            warm_out = dram.tile([2, 128], FP8, tag="warm_out")
            nc.gpsimd.collective_compute(
                "AllGather",
                mybir.AluOpType.bypass,
                replica_groups=[[0, 1], [2, 3], [4, 5], [6, 7]],
                ins=[warm_in[:].opt()],
                outs=[warm_out[:, :].opt()],
            )
            ident = persist.tile([128, 128], F32, tag="ident")
            nc.sync.dma_start(out=ident, in_=ident_in[:, :])
            mbcols = persist.tile([128, KT], F32, tag="mb")
            nc.gpsimd.dma_start(out=mbcols, in_=ap3(mbias, 0, [[1, 128], [128, KT]]))
            bqcol = persist.tile([128, 4], F32, tag="bq")
            nc.gpsimd.dma_start(out=bqcol, in_=ap3(bq, 0, [[1, 128], [128, 4]]))
            bkcol = persist.tile([128, 4], F32, tag="bk")
            nc.gpsimd.dma_start(out=bkcol, in_=ap3(bk, 0, [[1, 128], [128, 4]]))
            bv_bc = persist.tile([128, E], F32, tag="bvbc")
            nc.gpsimd.dma_start(out=bv_bc, in_=ap3(bv, 0, [[0, 128], [1, E]]))
            if apply_gb:
                ga_bc = persist.tile([128, E], F32, tag="gabc")
                be_bc = persist.tile([128, E], F32, tag="bebc")
                nc.gpsimd.dma_start(out=ga_bc, in_=ap3(gamma, 0, [[0, 128], [1, E]]))
                nc.gpsimd.dma_start(out=be_bc, in_=ap3(beta, 0, [[0, 128], [1, E]]))
            eps_t = persist.tile([128, 1], F32, tag="eps")
            nc.vector.memset(eps_t, EPS)

            # persistent fp8 operand tiles (paired [.., 2, ..] layouts)
            # w8[name]: [128e, 2(m), 2(i), 512f]; logical e = 256m + 128i + p
            w8 = {
                n: persist.tile([128, 2, 2, E], FP8, name=f"w8{n}", tag=f"w8{n}")
                for n in ("q", "k", "v")
            }
            # xq8[m][qc]: [128e, 2(i), 512s]
            xq8 = [
                [persist.tile([128, 2, 512], FP8, name=f"xq8_{m}_{c}", tag=f"xq8_{m}_{c}") for c in range(QC)]
                for m in range(2)
            ]
            # xkv8[m]: [128e, 2(i), S_OWN]
            xkv8 = [persist.tile([128, 2, S_OWN], FP8, name=f"xkv8_{m}", tag=f"xkv8_{m}") for m in range(2)]
            # qT8[m][qc]: [128f, 2(i), 512q]   (f = 256m + 128i + p)
            qT8 = [
                [persist.tile([128, 2, 512], FP8, name=f"qT8_{m}_{c}", tag=f"qT8_{m}_{c}") for c in range(QC)]
                for m in range(2)
            ]
            # kT8[m]: [128f, 2(i), 2*S_OWN]  (k cols: [own | sibling])
            kT8 = [persist.tile([128, 2, 2 * S_OWN], FP8, name=f"kT8_{m}", tag=f"kT8_{m}") for m in range(2)]
            # v8[jp]: [128k, 2(i), 513]  (k = 256jp + 128i + p; col 512 = ones)
            v8 = [persist.tile([128, 2, E + 1], FP8, name=f"v8_{j}", tag=f"v8_{j}") for j in range(JP)]
            # P tiles: pt[jp]: [128k, 4(qc), 2(i), 512q]
            pt = [persist.tile([128, 4, 2, 512], FP8, name=f"pt{j}", tag=f"pt{j}") for j in range(JP)]
            # residual x kept staged for the LN phase
            xqst = [persist.tile([128, 4, E], F32, name=f"xqst{c}", tag=f"xqst{c}") for c in range(QC)]

            # DRAM staging for the pair exchange
            ksz = [128 * 2 * (b - a) for a, b in chunks]
            vsz = 128 * 2 * (E + 1)
            jps_of = [list(range(a // 256, b // 256)) for a, b in chunks]
            ch_sz = [2 * k + len(j) * vsz for k, j in zip(ksz, jps_of)]
            NCH = len(chunks)
            kv_in = [
                dram.tile([ch_sz[c]], FP8, name=f"kv_in{c}", tag=f"kv_in{c}")
                for c in range(NCH)
            ]
            kv_out = [
                dram.tile([2, ch_sz[c]], FP8, name=f"kv_out{c}", tag=f"kv_out{c}")
                for c in range(NCH)
            ]
            groups = [[0, 1], [2, 3], [4, 5], [6, 7]]
            sib = 1 - (nc.gpsimd.partition_id() & 1)

            with (
                tc.tile_pool(name="wst", bufs=1) as wstp,
                tc.tile_pool(name="xst", bufs=3) as xstp,
                tc.tile_pool(name="tp", bufs=2, space="PSUM") as tpp,
                tc.tile_pool(name="pr", bufs=4, space="PSUM") as prp,
            ):
                def load_x(src, a, nt, xst):
                    """DMA rows a..a+nt*128 of src into xst[:, 0:nt, :]."""
                    nc.sync.dma_start(
                        out=xst[:, 0:nt, :],
                        in_=ap3(src, a * E, [[512, 128], [65536, nt], [1, 512]]),
                    )

                # interleave the bulk loads so chunk-0 compute starts ASAP:
                # xkv0 | Wk | xkv1 | Wv | xkv2  (Wq + xq stream later, during
                # the kv-chunk compute)
                wst = {}
                xsts = []
                wseq = [("k", WkT), ("v", WvT)]
                for c, (a, b) in enumerate(chunks):
                    xst = xstp.tile([128, 4, E], F32, tag="xst")
                    load_x(xkv, a, (b - a) // 128, xst)
                    xsts.append(xst)
                    if c < len(wseq):
                        name, wdram = wseq[c]
                        t = wstp.tile(
                            [128, 4, E], F32, name=f"wst{name}", tag=f"wst{name}"
                        )
                        nc.sync.dma_start(
                            out=t,
                            in_=ap3(wdram, 0, [[512, 128], [65536, 4], [1, 512]]),
                        )
                        wst[name] = t
                        nc.scalar.copy(out=w8[name], in_=t)
                for name, wdram in wseq[len(chunks) :]:
                    t = wstp.tile([128, 4, E], F32, name=f"wst{name}", tag=f"wst{name}")
                    nc.sync.dma_start(
                        out=t, in_=ap3(wdram, 0, [[512, 128], [65536, 4], [1, 512]])
                    )
                    wst[name] = t
                    nc.scalar.copy(out=w8[name], in_=t)
                wst["q"] = wstp.tile([128, 4, E], F32, name="wstq", tag="wstq")
                nc.scalar.dma_start(
                    out=wst["q"],
                    in_=ap3(WqT, 0, [[512, 128], [65536, 4], [1, 512]]),
                )
                for qc in range(QC):
                    nc.scalar.dma_start(
                        out=xqst[qc][:, 0:4, :],
                        in_=ap3(
                            xq, qc * 512 * E, [[512, 128], [65536, 4], [1, 512]]
                        ),
                    )

                def transpose_chunk(xst, nt, wr):
                    """xst [128,nt,512] f32 -> wr(et, tp_ap) for each e-tile."""
                    for et in range(4):
                        tp = tpp.tile([128, 512], F32, tag="tp")
                        for st in range(nt):
                            nc.tensor.transpose(
                                tp[:, st * 128 : (st + 1) * 128],
                                xst[:, st, et * 128 : (et + 1) * 128],
                                ident,
                            )
                        wr(et, tp[:, 0 : nt * 128])

                # ---- KV chunks first: project, ship, gather (pipelined) ----
                for c, (a, b) in enumerate(chunks):
                    nt = (b - a) // 128
                    transpose_chunk(
                        xsts[c],
                        nt,
                        lambda et, tp_ap, a=a, nt=nt: nc.vector.tensor_copy(
                            xkv8[et // 2][:, et % 2, a : a + nt * 128], tp_ap
                        ),
                    )
                    # K^T [f, s]: psum tile per f-tile ft
                    for ft in range(4):
                        ps = prp.tile([128, 512], F32, tag="pr")
                        for m in range(2):
                            nc.tensor.matmul(
                                ps[:, 0 : b - a],
                                w8["k"][:, m, :, ft * 128 : (ft + 1) * 128],
                                xkv8[m][:, :, a:b],
                                start=(m == 0),
                                stop=(m == 1),
                                perf_mode=DR,
                            )
                        nc.vector.tensor_scalar_add(
                            kT8[ft // 2][:, ft % 2, a:b],
                            ps[:, 0 : b - a],
                            bkcol[:, ft : ft + 1],
                        )
                    # V [s, f] per k-tile t
                    for t in range(a // 128, b // 128):
                        ps = prp.tile([128, 512], F32, tag="pr")
                        for m in range(2):
                            nc.tensor.matmul(
                                ps,
                                xkv8[m][:, :, t * 128 : (t + 1) * 128],
                                w8["v"][:, m, :, :],
                                start=(m == 0),
                                stop=(m == 1),
                                perf_mode=DR,
                            )
                        nc.vector.tensor_add(v8[t // 2][:, t % 2, 0:E], ps, bv_bc)
                        if t % 2 == 1:
                            nc.vector.memset(v8[t // 2][:, :, E : E + 1], 1.0)
                    # ship own K^T/V chunk + 2-core AllGather
                    for m in range(2):
                        nc.gpsimd.dma_start(
                            out=kv_in[c][m * ksz[c] : (m + 1) * ksz[c]],
                            in_=kT8[m][:, :, a:b],
                        )
                    for j, jp in enumerate(jps_of[c]):
                        o = 2 * ksz[c] + j * vsz
                        nc.gpsimd.dma_start(out=kv_in[c][o : o + vsz], in_=v8[jp])
                    nc.gpsimd.collective_compute(
                        "AllGather",
                        mybir.AluOpType.bypass,
                        replica_groups=groups,
                        ins=[kv_in[c][:].opt()],
                        outs=[kv_out[c][:, :].opt()],
                    )
                    # sibling half: load from the other core's gather slot
                    for m in range(2):
                        nc.gpsimd.dma_start(
                            out=kT8[m][:, :, S_OWN + a : S_OWN + b],
                            in_=kv_out[c][bass.ds(sib, 1), m * ksz[c] : (m + 1) * ksz[c]],
                        )
                    for j, jp in enumerate(jps_of[c]):
                        o = 2 * ksz[c] + j * vsz
                        nc.gpsimd.dma_start(
                            out=v8[JPO + jp], in_=kv_out[c][bass.ds(sib, 1), o : o + vsz]
                        )

                # ---- Q: transpose + project per q-chunk ----
                nc.scalar.copy(out=w8["q"], in_=wst["q"])
                for qc in range(QC):
                    transpose_chunk(
                        xqst[qc],
                        4,
                        lambda et, tp_ap, qc=qc: nc.scalar.copy(
                            out=xq8[et // 2][qc][:, et % 2, :], in_=tp_ap
                        ),
                    )
                    for ft in range(4):
                        ps = prp.tile([128, 512], F32, tag="pr")
                        for m in range(2):
                            nc.tensor.matmul(
                                ps,
                                w8["q"][:, m, :, ft * 128 : (ft + 1) * 128],
                                xq8[m][qc],
                                start=(m == 0),
                                stop=(m == 1),
                                perf_mode=DR,
                            )
                        nc.vector.tensor_scalar_add(
                            qT8[ft // 2][qc][:, ft % 2, :],
                            ps,
                            bqcol[:, ft : ft + 1],
                        )

            # ---------------- scores + exp (all k-tiles) ----------------
            with tc.tile_pool(name="sc", bufs=2, space="PSUM") as scp:
                for kt in range(KT):
                    sc = scp.tile([128, 4, 512], F32, tag="sc")
                    for m in range(2):
                        for qc in range(QC):
                            nc.tensor.matmul(
                                sc[:, qc, :],
                                kT8[m][:, :, kt * 128 : (kt + 1) * 128],
                                qT8[m][qc],
                                start=(m == 0),
                                stop=(m == 1),
                                perf_mode=DR,
                            )
                    nc.scalar.activation(
                        out=pt[kt // 2][:, :, kt % 2, :],
                        in_=sc,
                        func=AF.Exp,
                        bias=mbcols[:, kt : kt + 1],
                        scale=SCALE,
                    )

            # ---------------- ctx + residual + layernorm ----------------
            with (
                tc.tile_pool(name="cx", bufs=4, space="PSUM") as cxp,
                tc.tile_pool(name="wk", bufs=4) as work,
            ):
                for qi in range(16):
                    qc, st = qi // 4, qi % 4
                    cs = cxp.tile([128, 2, 512], F32, tag="cs")
                    for jp in range(JP):
                        lhs = pt[jp][:, qc, :, st * 128 : (st + 1) * 128]
                        nc.tensor.matmul(
                            cs[:, 0, 0:256],
                            lhs,
                            v8[jp][:, :, 0:256],
                            start=(jp == 0),
                            stop=(jp == JP - 1),
                            perf_mode=DR,
                        )
                        nc.tensor.matmul(
                            cs[:, 1, 0:257],
                            lhs,
                            v8[jp][:, :, 256 : E + 1],
                            start=(jp == 0),
                            stop=(jp == JP - 1),
                            perf_mode=DR,
                        )
                    recip = work.tile([128, 1], F32, tag="recip")
                    nc.vector.reciprocal(recip, cs[:, 1, 256:257])
                    h = work.tile([128, E], F32, tag="h")
                    nc.vector.scalar_tensor_tensor(
                        out=h[:, 0:256],
                        in0=cs[:, 0, 0:256],
                        scalar=recip,
                        in1=xqst[qc][:, st, 0:256],
                        op0=OP.mult,
                        op1=OP.add,
                    )
                    nc.vector.scalar_tensor_tensor(
                        out=h[:, 256:512],
                        in0=cs[:, 1, 0:256],
                        scalar=recip,
                        in1=xqst[qc][:, st, 256:512],
                        op0=OP.mult,
                        op1=OP.add,
                    )
                    st6 = work.tile([128, 6], F32, tag="st6")
                    nc.vector.bn_stats(out=st6, in_=h)
                    mv = work.tile([128, 2], F32, tag="mv")
                    nc.vector.bn_aggr(out=mv, in_=st6)
                    std = work.tile([128, 1], F32, tag="std")
                    nc.scalar.activation(
                        out=std, in_=mv[:, 1:2], func=AF.Sqrt, bias=eps_t
                    )
                    rstd = work.tile([128, 1], F32, tag="rstd")
                    nc.vector.reciprocal(rstd, std)
                    nmu = work.tile([128, 1], F32, tag="nmu")
                    nc.vector.tensor_scalar(
                        out=nmu,
                        in0=mv[:, 0:1],
                        scalar1=rstd,
                        scalar2=-1.0,
                        op0=OP.mult,
                        op1=OP.mult,
                    )
                    o_t = work.tile([128, E], F32, tag="ot")
                    nc.scalar.activation(
                        out=o_t, in_=h, func=AF.Identity, bias=nmu, scale=rstd
                    )
                    if apply_gb:
                        nc.vector.tensor_mul(o_t, o_t, ga_bc)
                        nc.vector.tensor_add(o_t, o_t, be_bc)
                    nc.gpsimd.dma_start(
                        out=out[qi * 128 : (qi + 1) * 128, :], in_=o_t
                    )
    return nc


# test-harness knobs (the grading harness leaves these at defaults)
TRACE = False
LAST_RESULTS = None


def _ensure_axon_jax():
    """The Bass SPMD run goes through jax/PJRT on the axon platform. If the
    caller pinned jax to cpu (e.g. to run a reference model), unpin it and
    drop any initialized cpu-only backends."""
    import os

    import jax

    try:
        devs = jax.devices()
    except Exception:
        devs = []
    if any(d.platform not in ("cpu",) for d in devs):
        return
    os.environ.pop("JAX_PLATFORMS", None)
    try:
        jax.config.update("jax_platforms", None)
    except Exception:
        pass
    try:
        jax.clear_backends()
    except Exception:
        try:
            jax.extend.backend.clear_backends()
        except Exception:
            pass


def kernel(x, mask, Wq, bq, Wk, bk, Wv, bv, gamma, beta):
    global LAST_RESULTS
    _ensure_axon_jax()
    from concourse.bass_utils import run_bass_kernel_spmd

    x = np.ascontiguousarray(np.asarray(x, dtype=np.float32))
    mask = np.asarray(np.asarray(mask) != 0)
    # Masked keys get softmax weight exactly 0 (exp underflow), so attention
    # only needs the unmasked keys: pack them per core half, padded to a
    # 128 multiple (even tile count); pad slots get the -1e4 bias -> exp==0.
    counts = [
        int(mask[b, h * SQ : (h + 1) * SQ].sum()) for b in range(4) for h in range(2)
    ]
    nkt = max(2, -(-max(counts) // 128))
    nkt += nkt % 2
    S_OWN = nkt * 128
    common = {
        "WqT": np.ascontiguousarray(np.asarray(Wq, dtype=np.float32).T),
        "WkT": np.ascontiguousarray(np.asarray(Wk, dtype=np.float32).T),
        "WvT": np.ascontiguousarray(np.asarray(Wv, dtype=np.float32).T),
        "bq": np.ascontiguousarray(bq, dtype=np.float32),
        "bk": np.ascontiguousarray(bk, dtype=np.float32),
        "bv": np.ascontiguousarray(bv, dtype=np.float32),
        "gamma": np.ascontiguousarray(gamma, dtype=np.float32),
        "beta": np.ascontiguousarray(beta, dtype=np.float32),
        "ident": np.eye(128, dtype=np.float32),
    }

    def packed_kv(b, h):
        rows = x[b, h * SQ : (h + 1) * SQ]
        sel = rows[mask[b, h * SQ : (h + 1) * SQ]]
        xkv = np.zeros((S_OWN, E), dtype=np.float32)
        xkv[: len(sel)] = sel
        mb = np.full(S_OWN, MASK_NEG + SHIFT, dtype=np.float32)
        mb[: len(sel)] = SHIFT
        return xkv, mb

    in_maps = []
    for c in range(8):
        b, h = c // 2, c % 2
        xkv_own, mb_own = packed_kv(b, h)
        _, mb_sib = packed_kv(b, 1 - h)
        # key order inside the kernel is [own packed | sibling packed]
        in_maps.append(
            {
                "xq": np.ascontiguousarray(x[b, h * SQ : (h + 1) * SQ]),
                "xkv": xkv_own,
                "maskbias": np.concatenate([mb_own, mb_sib]),
                **common,
            }
        )
    apply_gb = not (
        np.all(np.asarray(gamma) == 1.0) and np.all(np.asarray(beta) == 0.0)
    )
    nc = build_nc(nkt, apply_gb)
    nc.compile()
    res = run_bass_kernel_spmd(nc, in_maps, core_ids=list(range(8)), trace=TRACE)
    LAST_RESULTS = res
    full = np.empty((4, S, E), dtype=np.float32)
    for c in range(8):
        b, h = c // 2, c % 2
        full[b, h * SQ : (h + 1) * SQ] = res.results[c]["out"]
    return full


# revision 15
# speedup vs baseline: 1.0072x; 1.0072x over previous
"""Fused single-head attention + residual + LayerNorm for Trainium2 (Bass/Tile).

Problem: B=4, S=4096, E=512 fp32.
  Q/K/V = x @ W^T + b ; S = QK^T/sqrt(E) ; mask keys ; softmax ; ctx = P@V ;
  out = LayerNorm(ctx + x) * gamma + beta

Sharding: 8 cores = 4 batches x 2 halves of the S=4096 rows. Each core
projects Q/K/V for its OWN 2048 rows only and runs attention + layernorm
for those rows; the two cores of a batch exchange their packed K^T/V with
per-chunk 2-core AllGathers pipelined behind the projections.

v2 strategy (vs the bf16 baseline):
  - All heavy matmuls in fp8e4 with MatmulPerfMode.DoubleRow: both
    operands are laid out [128, 2, free] so each matmul contracts 256
    rows at 0.5 PE cycles/row (2x+ the bf16 rate). The attention output
    ("context") is ~1.5% of the magnitude of the residual x, so fp8
    rounding in the attention path is strongly damped in the final
    output.
  - Masked keys are packed out at 128 granularity (not 512): nkt =
    ceil(max_unmasked/128) rounded even, so attention runs on 2*nkt
    k-tiles (20) instead of the 512-padded 24.
  - Scores are computed transposed S^T[k, q] (k on partitions) into one
    PSUM tile [128, 4qc, 512] per k-tile, so ONE ScalarE activation
    exp(s*scale + maskbias_k - 1) covers all 2048 q for that k-tile
    (the -1 shift keeps exp < 8 for fp8 headroom; softmax normalization
    cancels it). P is written directly in fp8 paired layout for ctx.
  - Row sums ride in the P@V matmul via a ones-column appended to V.
  - Single ctx accumulation over all 2*nkt k-tiles (no spill pass):
    the K/V pair exchange is hidden behind the projections + own-half
    scores. KV chunks are projected and gathered FIRST, before Q.
  - LayerNorm: h built by DVE scalar_tensor_tensor from PSUM, stats via
    bn_stats, sqrt on ScalarE (all sqrts happen after all exps -> one
    activation-table switch), normalize on GpSimd (SBUF-only engine).
"""

import sys

import numpy as np

sys.path.insert(0, "/opt/trn_rl_repo")

import concourse.bass as bass  # noqa: E402
import concourse.tile as tile  # noqa: E402
from concourse import bacc, mybir  # noqa: E402

E = 512
S = 4096  # keys per batch
SQ = 2048  # query rows per core
QC = SQ // 512  # 4   512-chunks along q
F32 = mybir.dt.float32
FP8 = mybir.dt.float8e4
SCALE = 1.0 / float(np.sqrt(E))
EPS = 1e-5
MASK_NEG = -10000.0
SHIFT = -1.0  # softmax-invariant score shift, keeps exp() in fp8 range
DR = mybir.MatmulPerfMode.DoubleRow


def build_nc(nkt, apply_gb):
    # nkt = own k-tiles of 128 packed (unmasked) keys, even
    assert nkt % 2 == 0
    S_OWN = nkt * 128
    KT = 2 * nkt  # total k-tiles (own + sibling)
    JP = nkt  # ctx pair-tiles of 256 keys
    JPO = nkt // 2  # own pair-tiles
    chunks = [(a, min(a + 512, S_OWN)) for a in range(0, S_OWN, 512)]

    nc = bacc.Bacc("TRN2", target_bir_lowering=False, debug=False)
    xq = nc.dram_tensor("xq", [SQ, E], F32, kind="ExternalInput")
    xkv = nc.dram_tensor("xkv", [S_OWN, E], F32, kind="ExternalInput")
    mbias = nc.dram_tensor("maskbias", [KT * 128], F32, kind="ExternalInput")
    WqT = nc.dram_tensor("WqT", [E, E], F32, kind="ExternalInput")
    WkT = nc.dram_tensor("WkT", [E, E], F32, kind="ExternalInput")
    WvT = nc.dram_tensor("WvT", [E, E], F32, kind="ExternalInput")
    bq = nc.dram_tensor("bq", [E], F32, kind="ExternalInput")
    bk = nc.dram_tensor("bk", [E], F32, kind="ExternalInput")
    bv = nc.dram_tensor("bv", [E], F32, kind="ExternalInput")
    gamma = nc.dram_tensor("gamma", [E], F32, kind="ExternalInput")
    beta = nc.dram_tensor("beta", [E], F32, kind="ExternalInput")
    ident_in = nc.dram_tensor("ident", [128, 128], F32, kind="ExternalInput")
    out = nc.dram_tensor("out", [SQ, E], F32, kind="ExternalOutput")

    AF = mybir.ActivationFunctionType
    OP = mybir.AluOpType

    def ap3(handle, offset, dims):
        a = handle[:]
        return bass.AP(tensor=a.tensor, offset=offset, ap=dims)

    with tile.TileContext(nc) as tc:
        with (
            tc.tile_pool(name="persist", bufs=1) as persist,
            tc.tile_pool(name="dram", bufs=1, space="DRAM") as dram,
        ):
            # ---------------- constants ----------------
            ident = persist.tile([128, 128], F32, tag="ident")
            nc.sync.dma_start(out=ident, in_=ident_in[:, :])
            mbcols = persist.tile([128, KT], F32, tag="mb")
            nc.gpsimd.dma_start(out=mbcols, in_=ap3(mbias, 0, [[1, 128], [128, KT]]))
            bqcol = persist.tile([128, 4], F32, tag="bq")
            nc.gpsimd.dma_start(out=bqcol, in_=ap3(bq, 0, [[1, 128], [128, 4]]))
            bkcol = persist.tile([128, 4], F32, tag="bk")
            nc.gpsimd.dma_start(out=bkcol, in_=ap3(bk, 0, [[1, 128], [128, 4]]))
            bv_bc = persist.tile([128, E], F32, tag="bvbc")
            nc.gpsimd.dma_start(out=bv_bc, in_=ap3(bv, 0, [[0, 128], [1, E]]))
            if apply_gb:
                ga_bc = persist.tile([128, E], F32, tag="gabc")
                be_bc = persist.tile([128, E], F32, tag="bebc")
                nc.gpsimd.dma_start(out=ga_bc, in_=ap3(gamma, 0, [[0, 128], [1, E]]))
                nc.gpsimd.dma_start(out=be_bc, in_=ap3(beta, 0, [[0, 128], [1, E]]))
            eps_t = persist.tile([128, 1], F32, tag="eps")
            nc.vector.memset(eps_t, EPS)

            # persistent fp8 operand tiles (paired [.., 2, ..] layouts)
            # w8[name]: [128e, 2(m), 2(i), 512f]; logical e = 256m + 128i + p
            w8 = {
                n: persist.tile([128, 2, 2, E], FP8, name=f"w8{n}", tag=f"w8{n}")
                for n in ("q", "k", "v")
            }
            # xq8[m][qc]: [128e, 2(i), 512s]
            xq8 = [
                [persist.tile([128, 2, 512], FP8, name=f"xq8_{m}_{c}", tag=f"xq8_{m}_{c}") for c in range(QC)]
                for m in range(2)
            ]
            # xkv8[m]: [128e, 2(i), S_OWN]
            xkv8 = [persist.tile([128, 2, S_OWN], FP8, name=f"xkv8_{m}", tag=f"xkv8_{m}") for m in range(2)]
            # qT8[m][qc]: [128f, 2(i), 512q]   (f = 256m + 128i + p)
            qT8 = [
                [persist.tile([128, 2, 512], FP8, name=f"qT8_{m}_{c}", tag=f"qT8_{m}_{c}") for c in range(QC)]
                for m in range(2)
            ]
            # kT8[m]: [128f, 2(i), 2*S_OWN]  (k cols: [own | sibling])
            kT8 = [persist.tile([128, 2, 2 * S_OWN], FP8, name=f"kT8_{m}", tag=f"kT8_{m}") for m in range(2)]
            # v8[jp]: [128k, 2(i), 513]  (k = 256jp + 128i + p; col 512 = ones)
            v8 = [persist.tile([128, 2, E + 1], FP8, name=f"v8_{j}", tag=f"v8_{j}") for j in range(JP)]
            # P tiles: pt[jp]: [128k, 4(qc), 2(i), 512q]
            pt = [persist.tile([128, 4, 2, 512], FP8, name=f"pt{j}", tag=f"pt{j}") for j in range(JP)]
            # residual x kept staged for the LN phase
            xqst = [persist.tile([128, 4, E], F32, name=f"xqst{c}", tag=f"xqst{c}") for c in range(QC)]

            # DRAM staging for the pair exchange
            ksz = [128 * 2 * (b - a) for a, b in chunks]
            vsz = 128 * 2 * (E + 1)
            jps_of = [list(range(a // 256, b // 256)) for a, b in chunks]
            ch_sz = [2 * k + len(j) * vsz for k, j in zip(ksz, jps_of)]
            NCH = len(chunks)
            kv_in = [
                dram.tile([ch_sz[c]], FP8, name=f"kv_in{c}", tag=f"kv_in{c}")
                for c in range(NCH)
            ]
            kv_out = [
                dram.tile([2, ch_sz[c]], FP8, name=f"kv_out{c}", tag=f"kv_out{c}")
                for c in range(NCH)
            ]
            groups = [[0, 1], [2, 3], [4, 5], [6, 7]]
            sib = 1 - (nc.gpsimd.partition_id() & 1)

            with (
                tc.tile_pool(name="wst", bufs=1) as wstp,
                tc.tile_pool(name="xst", bufs=3) as xstp,
                tc.tile_pool(name="tp", bufs=2, space="PSUM") as tpp,
                tc.tile_pool(name="pr", bufs=4, space="PSUM") as prp,
            ):
                def load_x(src, a, nt, xst):
                    """DMA rows a..a+nt*128 of src into xst[:, 0:nt, :]."""
                    nc.sync.dma_start(
                        out=xst[:, 0:nt, :],
                        in_=ap3(src, a * E, [[512, 128], [65536, nt], [1, 512]]),
                    )

                # interleave the bulk loads so chunk-0 compute starts ASAP:
                # xkv0 | Wk | xkv1 | Wv | xkv2  (Wq + xq stream later, during
                # the kv-chunk compute)
                wst = {}
                xsts = []
                wseq = [("k", WkT), ("v", WvT)]
                for c, (a, b) in enumerate(chunks):
                    xst = xstp.tile([128, 4, E], F32, tag="xst")
                    load_x(xkv, a, (b - a) // 128, xst)
                    xsts.append(xst)
                    if c < len(wseq):
                        name, wdram = wseq[c]
                        t = wstp.tile(
                            [128, 4, E], F32, name=f"wst{name}", tag=f"wst{name}"
                        )
                        nc.sync.dma_start(
                            out=t,
                            in_=ap3(wdram, 0, [[512, 128], [65536, 4], [1, 512]]),
                        )
                        wst[name] = t
                        nc.scalar.copy(out=w8[name], in_=t)
                for name, wdram in wseq[len(chunks) :]:
                    t = wstp.tile([128, 4, E], F32, name=f"wst{name}", tag=f"wst{name}")
                    nc.sync.dma_start(
                        out=t, in_=ap3(wdram, 0, [[512, 128], [65536, 4], [1, 512]])
                    )
                    wst[name] = t
                    nc.scalar.copy(out=w8[name], in_=t)
                wst["q"] = wstp.tile([128, 4, E], F32, name="wstq", tag="wstq")
                nc.scalar.dma_start(
                    out=wst["q"],
                    in_=ap3(WqT, 0, [[512, 128], [65536, 4], [1, 512]]),
                )
                for qc in range(QC):
                    nc.scalar.dma_start(
                        out=xqst[qc][:, 0:4, :],
                        in_=ap3(
                            xq, qc * 512 * E, [[512, 128], [65536, 4], [1, 512]]
                        ),
                    )

                def transpose_chunk(xst, nt, wr):
                    """xst [128,nt,512] f32 -> wr(et, tp_ap) for each e-tile."""
                    for et in range(4):
                        tp = tpp.tile([128, 512], F32, tag="tp")
                        for st in range(nt):
                            nc.tensor.transpose(
                                tp[:, st * 128 : (st + 1) * 128],
                                xst[:, st, et * 128 : (et + 1) * 128],
                                ident,
                            )
                        wr(et, tp[:, 0 : nt * 128])

                # ---- KV chunks first: project, ship, gather (pipelined) ----
                for c, (a, b) in enumerate(chunks):
                    nt = (b - a) // 128
                    transpose_chunk(
                        xsts[c],
                        nt,
                        lambda et, tp_ap, a=a, nt=nt: nc.vector.tensor_copy(
                            xkv8[et // 2][:, et % 2, a : a + nt * 128], tp_ap
                        ),
                    )
                    # K^T [f, s]: psum tile per f-tile ft
                    for ft in range(4):
                        ps = prp.tile([128, 512], F32, tag="pr")
                        for m in range(2):
                            nc.tensor.matmul(
                                ps[:, 0 : b - a],
                                w8["k"][:, m, :, ft * 128 : (ft + 1) * 128],
                                xkv8[m][:, :, a:b],
                                start=(m == 0),
                                stop=(m == 1),
                                perf_mode=DR,
                            )
                        nc.vector.tensor_scalar_add(
                            kT8[ft // 2][:, ft % 2, a:b],
                            ps[:, 0 : b - a],
                            bkcol[:, ft : ft + 1],
                        )
                    # V [s, f] per k-tile t
                    for t in range(a // 128, b // 128):
                        ps = prp.tile([128, 512], F32, tag="pr")
                        for m in range(2):
                            nc.tensor.matmul(
                                ps,
                                xkv8[m][:, :, t * 128 : (t + 1) * 128],
                                w8["v"][:, m, :, :],
                                start=(m == 0),
                                stop=(m == 1),
                                perf_mode=DR,
                            )
                        nc.vector.tensor_add(v8[t // 2][:, t % 2, 0:E], ps, bv_bc)
                        if t % 2 == 1:
                            nc.vector.memset(v8[t // 2][:, :, E : E + 1], 1.0)
                    # ship own K^T/V chunk + 2-core AllGather
                    for m in range(2):
                        nc.sync.dma_start(
                            out=kv_in[c][m * ksz[c] : (m + 1) * ksz[c]],
                            in_=kT8[m][:, :, a:b],
                        )
                    for j, jp in enumerate(jps_of[c]):
                        o = 2 * ksz[c] + j * vsz
                        nc.sync.dma_start(out=kv_in[c][o : o + vsz], in_=v8[jp])
                    nc.gpsimd.collective_compute(
                        "AllGather",
                        mybir.AluOpType.bypass,
                        replica_groups=groups,
                        ins=[kv_in[c][:].opt()],
                        outs=[kv_out[c][:, :].opt()],
                    )
                # sibling halves: after ALL gather triggers are dispatched
                # (so no sib-load wait delays a later gather's CC dispatch)
                for c, (a, b) in enumerate(chunks):
                    for m in range(2):
                        nc.gpsimd.dma_start(
                            out=kT8[m][:, :, S_OWN + a : S_OWN + b],
                            in_=kv_out[c][bass.ds(sib, 1), m * ksz[c] : (m + 1) * ksz[c]],
                        )
                    for j, jp in enumerate(jps_of[c]):
                        o = 2 * ksz[c] + j * vsz
                        nc.gpsimd.dma_start(
                            out=v8[JPO + jp], in_=kv_out[c][bass.ds(sib, 1), o : o + vsz]
                        )

                # ---- Q: transpose + project per q-chunk ----
                nc.scalar.copy(out=w8["q"], in_=wst["q"])
                for qc in range(QC):
                    transpose_chunk(
                        xqst[qc],
                        4,
                        lambda et, tp_ap, qc=qc: nc.scalar.copy(
                            out=xq8[et // 2][qc][:, et % 2, :], in_=tp_ap
                        ),
                    )
                    for ft in range(4):
                        ps = prp.tile([128, 512], F32, tag="pr")
                        for m in range(2):
                            nc.tensor.matmul(
                                ps,
                                w8["q"][:, m, :, ft * 128 : (ft + 1) * 128],
                                xq8[m][qc],
                                start=(m == 0),
                                stop=(m == 1),
                                perf_mode=DR,
                            )
                        nc.vector.tensor_scalar_add(
                            qT8[ft // 2][qc][:, ft % 2, :],
                            ps,
                            bqcol[:, ft : ft + 1],
                        )

            # ---------------- scores + exp (all k-tiles) ----------------
            with tc.tile_pool(name="sc", bufs=2, space="PSUM") as scp:
                for kt in range(KT):
                    sc = scp.tile([128, 4, 512], F32, tag="sc")
                    for m in range(2):
                        for qc in range(QC):
                            nc.tensor.matmul(
                                sc[:, qc, :],
                                kT8[m][:, :, kt * 128 : (kt + 1) * 128],
                                qT8[m][qc],
                                start=(m == 0),
                                stop=(m == 1),
                                perf_mode=DR,
                            )
                    nc.scalar.activation(
                        out=pt[kt // 2][:, :, kt % 2, :],
                        in_=sc,
                        func=AF.Exp,
                        bias=mbcols[:, kt : kt + 1],
                        scale=SCALE,
                    )

            # ---------------- ctx + residual + layernorm ----------------
            with (
                tc.tile_pool(name="cx", bufs=4, space="PSUM") as cxp,
                tc.tile_pool(name="wk", bufs=4) as work,
            ):
                for qi in range(16):
                    qc, st = qi // 4, qi % 4
                    cs = cxp.tile([128, 2, 512], F32, tag="cs")
                    for jp in range(JP):
                        lhs = pt[jp][:, qc, :, st * 128 : (st + 1) * 128]
                        nc.tensor.matmul(
                            cs[:, 0, 0:256],
                            lhs,
                            v8[jp][:, :, 0:256],
                            start=(jp == 0),
                            stop=(jp == JP - 1),
                            perf_mode=DR,
                        )
                        nc.tensor.matmul(
                            cs[:, 1, 0:257],
                            lhs,
                            v8[jp][:, :, 256 : E + 1],
                            start=(jp == 0),
                            stop=(jp == JP - 1),
                            perf_mode=DR,
                        )
                    recip = work.tile([128, 1], F32, tag="recip")
                    nc.vector.reciprocal(recip, cs[:, 1, 256:257])
                    h = work.tile([128, E], F32, tag="h")
                    nc.vector.scalar_tensor_tensor(
                        out=h[:, 0:256],
                        in0=cs[:, 0, 0:256],
                        scalar=recip,
                        in1=xqst[qc][:, st, 0:256],
                        op0=OP.mult,
                        op1=OP.add,
                    )
                    nc.vector.scalar_tensor_tensor(
                        out=h[:, 256:512],
                        in0=cs[:, 1, 0:256],
                        scalar=recip,
                        in1=xqst[qc][:, st, 256:512],
                        op0=OP.mult,
                        op1=OP.add,
                    )
                    st6 = work.tile([128, 6], F32, tag="st6")
                    nc.vector.bn_stats(out=st6, in_=h)
                    mv = work.tile([128, 2], F32, tag="mv")
                    nc.vector.bn_aggr(out=mv, in_=st6)
                    std = work.tile([128, 1], F32, tag="std")
                    nc.scalar.activation(
                        out=std, in_=mv[:, 1:2], func=AF.Sqrt, bias=eps_t
                    )
                    rstd = work.tile([128, 1], F32, tag="rstd")
                    nc.vector.reciprocal(rstd, std)
                    nmu = work.tile([128, 1], F32, tag="nmu")
                    nc.vector.tensor_scalar(
                        out=nmu,
                        in0=mv[:, 0:1],
                        scalar1=rstd,
                        scalar2=-1.0,
                        op0=OP.mult,
                        op1=OP.mult,
                    )
                    o_t = work.tile([128, E], F32, tag="ot")
                    nc.scalar.activation(
                        out=o_t, in_=h, func=AF.Identity, bias=nmu, scale=rstd
                    )
                    if apply_gb:
                        nc.vector.tensor_mul(o_t, o_t, ga_bc)
                        nc.vector.tensor_add(o_t, o_t, be_bc)
                    nc.gpsimd.dma_start(
                        out=out[qi * 128 : (qi + 1) * 128, :], in_=o_t
                    )
    return nc


# test-harness knobs (the grading harness leaves these at defaults)
TRACE = False
LAST_RESULTS = None


def _ensure_axon_jax():
    """The Bass SPMD run goes through jax/PJRT on the axon platform. If the
    caller pinned jax to cpu (e.g. to run a reference model), unpin it and
    drop any initialized cpu-only backends."""
    import os

    import jax

    try:
        devs = jax.devices()
    except Exception:
        devs = []
    if any(d.platform not in ("cpu",) for d in devs):
        return
    os.environ.pop("JAX_PLATFORMS", None)
    try:
        jax.config.update("jax_platforms", None)
    except Exception:
        pass
    try:
        jax.clear_backends()
    except Exception:
        try:
            jax.extend.backend.clear_backends()
        except Exception:
            pass


def kernel(x, mask, Wq, bq, Wk, bk, Wv, bv, gamma, beta):
    global LAST_RESULTS
    _ensure_axon_jax()
    from concourse.bass_utils import run_bass_kernel_spmd

    x = np.ascontiguousarray(np.asarray(x, dtype=np.float32))
    mask = np.asarray(np.asarray(mask) != 0)
    # Masked keys get softmax weight exactly 0 (exp underflow), so attention
    # only needs the unmasked keys: pack them per core half, padded to a
    # 128 multiple (even tile count); pad slots get the -1e4 bias -> exp==0.
    counts = [
        int(mask[b, h * SQ : (h + 1) * SQ].sum()) for b in range(4) for h in range(2)
    ]
    nkt = max(2, -(-max(counts) // 128))
    nkt += nkt % 2
    S_OWN = nkt * 128
    common = {
        "WqT": np.ascontiguousarray(np.asarray(Wq, dtype=np.float32).T),
        "WkT": np.ascontiguousarray(np.asarray(Wk, dtype=np.float32).T),
        "WvT": np.ascontiguousarray(np.asarray(Wv, dtype=np.float32).T),
        "bq": np.ascontiguousarray(bq, dtype=np.float32),
        "bk": np.ascontiguousarray(bk, dtype=np.float32),
        "bv": np.ascontiguousarray(bv, dtype=np.float32),
        "gamma": np.ascontiguousarray(gamma, dtype=np.float32),
        "beta": np.ascontiguousarray(beta, dtype=np.float32),
        "ident": np.eye(128, dtype=np.float32),
    }

    def packed_kv(b, h):
        rows = x[b, h * SQ : (h + 1) * SQ]
        sel = rows[mask[b, h * SQ : (h + 1) * SQ]]
        xkv = np.zeros((S_OWN, E), dtype=np.float32)
        xkv[: len(sel)] = sel
        mb = np.full(S_OWN, MASK_NEG + SHIFT, dtype=np.float32)
        mb[: len(sel)] = SHIFT
        return xkv, mb

    in_maps = []
    for c in range(8):
        b, h = c // 2, c % 2
        xkv_own, mb_own = packed_kv(b, h)
        _, mb_sib = packed_kv(b, 1 - h)
        # key order inside the kernel is [own packed | sibling packed]
        in_maps.append(
            {
                "xq": np.ascontiguousarray(x[b, h * SQ : (h + 1) * SQ]),
                "xkv": xkv_own,
                "maskbias": np.concatenate([mb_own, mb_sib]),
                **common,
            }
        )
    apply_gb = not (
        np.all(np.asarray(gamma) == 1.0) and np.all(np.asarray(beta) == 0.0)
    )
    nc = build_nc(nkt, apply_gb)
    nc.compile()
    res = run_bass_kernel_spmd(nc, in_maps, core_ids=list(range(8)), trace=TRACE)
    LAST_RESULTS = res
    full = np.empty((4, S, E), dtype=np.float32)
    for c in range(8):
        b, h = c // 2, c % 2
        full[b, h * SQ : (h + 1) * SQ] = res.results[c]["out"]
    return full


# revision 16
# speedup vs baseline: 1.1839x; 1.1754x over previous
"""Fused single-head attention + residual + LayerNorm for Trainium2 (Bass/Tile).

Problem: B=4, S=4096, E=512 fp32.
  Q/K/V = x @ W^T + b ; S = QK^T/sqrt(E) ; mask keys ; softmax ; ctx = P@V ;
  out = LayerNorm(ctx + x) * gamma + beta

Sharding: 8 cores = 4 batches x 2 halves of the S=4096 rows. Each core
projects Q/K/V for its OWN 2048 rows only and runs attention + layernorm
for those rows; the two cores of a batch exchange their packed K^T/V with
per-chunk 2-core AllGathers pipelined behind the projections.

v2 strategy (vs the bf16 baseline):
  - All heavy matmuls in fp8e4 with MatmulPerfMode.DoubleRow: both
    operands are laid out [128, 2, free] so each matmul contracts 256
    rows at 0.5 PE cycles/row (2x+ the bf16 rate). The attention output
    ("context") is ~1.5% of the magnitude of the residual x, so fp8
    rounding in the attention path is strongly damped in the final
    output.
  - Masked keys are packed out at 128 granularity (not 512): nkt =
    ceil(max_unmasked/128) rounded even, so attention runs on 2*nkt
    k-tiles (20) instead of the 512-padded 24.
  - Scores are computed transposed S^T[k, q] (k on partitions) into one
    PSUM tile [128, 4qc, 512] per k-tile, so ONE ScalarE activation
    exp(s*scale + maskbias_k - 1) covers all 2048 q for that k-tile
    (the -1 shift keeps exp < 8 for fp8 headroom; softmax normalization
    cancels it). P is written directly in fp8 paired layout for ctx.
  - Row sums ride in the P@V matmul via a ones-column appended to V.
  - Single ctx accumulation over all 2*nkt k-tiles (no spill pass):
    the K/V pair exchange is hidden behind the projections + own-half
    scores. KV chunks are projected and gathered FIRST, before Q.
  - LayerNorm: h built by DVE scalar_tensor_tensor from PSUM, stats via
    bn_stats, sqrt on ScalarE (all sqrts happen after all exps -> one
    activation-table switch), normalize on GpSimd (SBUF-only engine).
"""

import sys

import numpy as np

sys.path.insert(0, "/opt/trn_rl_repo")

import concourse.bass as bass  # noqa: E402
import concourse.tile as tile  # noqa: E402
from concourse import bacc, mybir  # noqa: E402

E = 512
S = 4096  # keys per batch
SQ = 2048  # query rows per core
QC = SQ // 512  # 4   512-chunks along q
F32 = mybir.dt.float32
FP8 = mybir.dt.float8e4
SCALE = 1.0 / float(np.sqrt(E))
EPS = 1e-5
MASK_NEG = -10000.0
SHIFT = -1.0  # softmax-invariant score shift, keeps exp() in fp8 range
DR = mybir.MatmulPerfMode.DoubleRow


def build_nc(nkt, apply_gb):
    # nkt = own k-tiles of 128 packed (unmasked) keys, even
    assert nkt % 2 == 0
    S_OWN = nkt * 128
    KT = 2 * nkt  # total k-tiles (own + sibling)
    JP = nkt  # ctx pair-tiles of 256 keys
    JPO = nkt // 2  # own pair-tiles
    chunks = [(a, min(a + 512, S_OWN)) for a in range(0, S_OWN, 512)]

    nc = bacc.Bacc("TRN2", target_bir_lowering=False, debug=False)
    xq = nc.dram_tensor("xq", [SQ, E], F32, kind="ExternalInput")
    xkv = nc.dram_tensor("xkv", [S_OWN, E], F32, kind="ExternalInput")
    mbias = nc.dram_tensor("maskbias", [KT * 128], F32, kind="ExternalInput")
    WqT = nc.dram_tensor("WqT", [E, E], F32, kind="ExternalInput")
    WkT = nc.dram_tensor("WkT", [E, E], F32, kind="ExternalInput")
    WvT = nc.dram_tensor("WvT", [E, E], F32, kind="ExternalInput")
    bq = nc.dram_tensor("bq", [E], F32, kind="ExternalInput")
    bk = nc.dram_tensor("bk", [E], F32, kind="ExternalInput")
    bv = nc.dram_tensor("bv", [E], F32, kind="ExternalInput")
    gamma = nc.dram_tensor("gamma", [E], F32, kind="ExternalInput")
    beta = nc.dram_tensor("beta", [E], F32, kind="ExternalInput")
    ident_in = nc.dram_tensor("ident", [128, 128], F32, kind="ExternalInput")
    out = nc.dram_tensor("out", [SQ, E], F32, kind="ExternalOutput")

    AF = mybir.ActivationFunctionType
    OP = mybir.AluOpType

    def ap3(handle, offset, dims):
        a = handle[:]
        return bass.AP(tensor=a.tensor, offset=offset, ap=dims)

    with tile.TileContext(nc) as tc:
        with (
            tc.tile_pool(name="persist", bufs=1) as persist,
            tc.tile_pool(name="dram", bufs=1, space="DRAM") as dram,
        ):
            # ---------------- constants ----------------
            ident = persist.tile([128, 128], F32, tag="ident")
            nc.sync.dma_start(out=ident, in_=ident_in[:, :])
            mbcols = persist.tile([128, KT], F32, tag="mb")
            nc.gpsimd.dma_start(out=mbcols, in_=ap3(mbias, 0, [[1, 128], [128, KT]]))
            bqcol = persist.tile([128, 4], F32, tag="bq")
            nc.gpsimd.dma_start(out=bqcol, in_=ap3(bq, 0, [[1, 128], [128, 4]]))
            bkcol = persist.tile([128, 4], F32, tag="bk")
            nc.gpsimd.dma_start(out=bkcol, in_=ap3(bk, 0, [[1, 128], [128, 4]]))
            bv_bc = persist.tile([128, E], F32, tag="bvbc")
            nc.gpsimd.dma_start(out=bv_bc, in_=ap3(bv, 0, [[0, 128], [1, E]]))
            if apply_gb:
                ga_bc = persist.tile([128, E], F32, tag="gabc")
                be_bc = persist.tile([128, E], F32, tag="bebc")
                nc.gpsimd.dma_start(out=ga_bc, in_=ap3(gamma, 0, [[0, 128], [1, E]]))
                nc.gpsimd.dma_start(out=be_bc, in_=ap3(beta, 0, [[0, 128], [1, E]]))
            eps_t = persist.tile([128, 1], F32, tag="eps")
            nc.vector.memset(eps_t, EPS)

            # persistent fp8 operand tiles (paired [.., 2, ..] layouts)
            # w8[name]: [128e, 2(m), 2(i), 512f]; logical e = 256m + 128i + p
            w8 = {
                n: persist.tile([128, 2, 2, E], FP8, name=f"w8{n}", tag=f"w8{n}")
                for n in ("q", "k", "v")
            }
            # xq8[m][qc]: [128e, 2(i), 512s]
            xq8 = [
                [persist.tile([128, 2, 512], FP8, name=f"xq8_{m}_{c}", tag=f"xq8_{m}_{c}") for c in range(QC)]
                for m in range(2)
            ]
            # xkv8[m]: [128e, 2(i), S_OWN]
            xkv8 = [persist.tile([128, 2, S_OWN], FP8, name=f"xkv8_{m}", tag=f"xkv8_{m}") for m in range(2)]
            # qT8[m][qc]: [128f, 2(i), 512q]   (f = 256m + 128i + p)
            qT8 = [
                [persist.tile([128, 2, 512], FP8, name=f"qT8_{m}_{c}", tag=f"qT8_{m}_{c}") for c in range(QC)]
                for m in range(2)
            ]
            # kT8[m]: [128f, 2(i), 2*S_OWN]  (k cols: [own | sibling])
            kT8 = [persist.tile([128, 2, 2 * S_OWN], FP8, name=f"kT8_{m}", tag=f"kT8_{m}") for m in range(2)]
            # v8[jp]: [128k, 2(i), 513]  (k = 256jp + 128i + p; col 512 = ones)
            v8 = [persist.tile([128, 2, E + 1], FP8, name=f"v8_{j}", tag=f"v8_{j}") for j in range(JP)]
            # P tiles: pt[jp]: [128k, 4(qc), 2(i), 512q]
            pt = [persist.tile([128, 4, 2, 512], FP8, name=f"pt{j}", tag=f"pt{j}") for j in range(JP)]
            # residual x kept staged for the LN phase
            xqst = [persist.tile([128, 4, E], F32, name=f"xqst{c}", tag=f"xqst{c}") for c in range(QC)]

            # DRAM staging for the pair exchange
            ksz = [128 * 2 * (b - a) for a, b in chunks]
            vsz = 128 * 2 * (E + 1)
            jps_of = [list(range(a // 256, b // 256)) for a, b in chunks]
            ch_sz = [2 * k + len(j) * vsz for k, j in zip(ksz, jps_of)]
            NCH = len(chunks)
            kv_in = [
                dram.tile([ch_sz[c]], FP8, name=f"kv_in{c}", tag=f"kv_in{c}")
                for c in range(NCH)
            ]
            kv_out = [
                dram.tile([2, ch_sz[c]], FP8, name=f"kv_out{c}", tag=f"kv_out{c}")
                for c in range(NCH)
            ]
            groups = [[0, 1], [2, 3], [4, 5], [6, 7]]
            sib = 1 - (nc.gpsimd.partition_id() & 1)

            with (
                tc.tile_pool(name="wst", bufs=1) as wstp,
                tc.tile_pool(name="xst", bufs=3) as xstp,
                tc.tile_pool(name="tp", bufs=2, space="PSUM") as tpp,
                tc.tile_pool(name="pr", bufs=4, space="PSUM") as prp,
            ):
                def load_x(src, a, nt, xst):
                    """DMA rows a..a+nt*128 of src into xst[:, 0:nt, :]."""
                    nc.sync.dma_start(
                        out=xst[:, 0:nt, :],
                        in_=ap3(src, a * E, [[512, 128], [65536, nt], [1, 512]]),
                    )

                # interleave the bulk loads so chunk-0 compute starts ASAP:
                # xkv0 | Wk | xkv1 | Wv | xkv2  (Wq + xq stream later, during
                # the kv-chunk compute)
                wst = {}
                xsts = []
                wseq = [("k", WkT), ("v", WvT)]
                for c, (a, b) in enumerate(chunks):
                    xst = xstp.tile([128, 4, E], F32, tag="xst")
                    load_x(xkv, a, (b - a) // 128, xst)
                    xsts.append(xst)
                    if c < len(wseq):
                        name, wdram = wseq[c]
                        t = wstp.tile(
                            [128, 4, E], F32, name=f"wst{name}", tag=f"wst{name}"
                        )
                        nc.sync.dma_start(
                            out=t,
                            in_=ap3(wdram, 0, [[512, 128], [65536, 4], [1, 512]]),
                        )
                        wst[name] = t
                        nc.scalar.copy(out=w8[name], in_=t)
                for name, wdram in wseq[len(chunks) :]:
                    t = wstp.tile([128, 4, E], F32, name=f"wst{name}", tag=f"wst{name}")
                    nc.sync.dma_start(
                        out=t, in_=ap3(wdram, 0, [[512, 128], [65536, 4], [1, 512]])
                    )
                    wst[name] = t
                    nc.scalar.copy(out=w8[name], in_=t)
                wst["q"] = wstp.tile([128, 4, E], F32, name="wstq", tag="wstq")
                nc.sync.dma_start(
                    out=wst["q"],
                    in_=ap3(WqT, 0, [[512, 128], [65536, 4], [1, 512]]),
                )
                for qc in range(QC):
                    nc.sync.dma_start(
                        out=xqst[qc][:, 0:4, :],
                        in_=ap3(
                            xq, qc * 512 * E, [[512, 128], [65536, 4], [1, 512]]
                        ),
                    )

                def transpose_chunk(xst, nt, wr):
                    """xst [128,nt,512] f32 -> wr(et, tp_ap) for each e-tile."""
                    for et in range(4):
                        tp = tpp.tile([128, 512], F32, tag="tp")
                        for st in range(nt):
                            nc.tensor.transpose(
                                tp[:, st * 128 : (st + 1) * 128],
                                xst[:, st, et * 128 : (et + 1) * 128],
                                ident,
                            )
                        wr(et, tp[:, 0 : nt * 128])

                # ---- KV chunks first: project, ship, gather (pipelined) ----
                for c, (a, b) in enumerate(chunks):
                    nt = (b - a) // 128
                    transpose_chunk(
                        xsts[c],
                        nt,
                        lambda et, tp_ap, a=a, nt=nt: nc.vector.tensor_copy(
                            xkv8[et // 2][:, et % 2, a : a + nt * 128], tp_ap
                        ),
                    )
                    # K^T [f, s]: psum tile per f-tile ft
                    for ft in range(4):
                        ps = prp.tile([128, 512], F32, tag="pr")
                        for m in range(2):
                            nc.tensor.matmul(
                                ps[:, 0 : b - a],
                                w8["k"][:, m, :, ft * 128 : (ft + 1) * 128],
                                xkv8[m][:, :, a:b],
                                start=(m == 0),
                                stop=(m == 1),
                                perf_mode=DR,
                            )
                        nc.vector.tensor_scalar_add(
                            kT8[ft // 2][:, ft % 2, a:b],
                            ps[:, 0 : b - a],
                            bkcol[:, ft : ft + 1],
                        )
                    # V [s, f] per k-tile t
                    for t in range(a // 128, b // 128):
                        ps = prp.tile([128, 512], F32, tag="pr")
                        for m in range(2):
                            nc.tensor.matmul(
                                ps,
                                xkv8[m][:, :, t * 128 : (t + 1) * 128],
                                w8["v"][:, m, :, :],
                                start=(m == 0),
                                stop=(m == 1),
                                perf_mode=DR,
                            )
                        nc.vector.tensor_add(v8[t // 2][:, t % 2, 0:E], ps, bv_bc)
                        if t % 2 == 1:
                            nc.vector.memset(v8[t // 2][:, :, E : E + 1], 1.0)
                    # ship own K^T/V chunk + 2-core AllGather
                    for m in range(2):
                        nc.sync.dma_start(
                            out=kv_in[c][m * ksz[c] : (m + 1) * ksz[c]],
                            in_=kT8[m][:, :, a:b],
                        )
                    for j, jp in enumerate(jps_of[c]):
                        o = 2 * ksz[c] + j * vsz
                        nc.sync.dma_start(out=kv_in[c][o : o + vsz], in_=v8[jp])
                    nc.gpsimd.collective_compute(
                        "AllGather",
                        mybir.AluOpType.bypass,
                        replica_groups=groups,
                        ins=[kv_in[c][:].opt()],
                        outs=[kv_out[c][:, :].opt()],
                    )
                # sibling halves: after ALL gather triggers are dispatched
                # (so no sib-load wait delays a later gather's CC dispatch)
                for c, (a, b) in enumerate(chunks):
                    for m in range(2):
                        nc.gpsimd.dma_start(
                            out=kT8[m][:, :, S_OWN + a : S_OWN + b],
                            in_=kv_out[c][bass.ds(sib, 1), m * ksz[c] : (m + 1) * ksz[c]],
                        )
                    for j, jp in enumerate(jps_of[c]):
                        o = 2 * ksz[c] + j * vsz
                        nc.gpsimd.dma_start(
                            out=v8[JPO + jp], in_=kv_out[c][bass.ds(sib, 1), o : o + vsz]
                        )

                # ---- Q: transpose + project per q-chunk ----
                nc.scalar.copy(out=w8["q"], in_=wst["q"])
                for qc in range(QC):
                    transpose_chunk(
                        xqst[qc],
                        4,
                        lambda et, tp_ap, qc=qc: nc.scalar.copy(
                            out=xq8[et // 2][qc][:, et % 2, :], in_=tp_ap
                        ),
                    )
                    for ft in range(4):
                        ps = prp.tile([128, 512], F32, tag="pr")
                        for m in range(2):
                            nc.tensor.matmul(
                                ps,
                                w8["q"][:, m, :, ft * 128 : (ft + 1) * 128],
                                xq8[m][qc],
                                start=(m == 0),
                                stop=(m == 1),
                                perf_mode=DR,
                            )
                        nc.vector.tensor_scalar_add(
                            qT8[ft // 2][qc][:, ft % 2, :],
                            ps,
                            bqcol[:, ft : ft + 1],
                        )

            # ---------------- scores + exp (all k-tiles) ----------------
            with tc.tile_pool(name="sc", bufs=2, space="PSUM") as scp:
                for kt in range(KT):
                    sc = scp.tile([128, 4, 512], F32, tag="sc")
                    for m in range(2):
                        for qc in range(QC):
                            nc.tensor.matmul(
                                sc[:, qc, :],
                                kT8[m][:, :, kt * 128 : (kt + 1) * 128],
                                qT8[m][qc],
                                start=(m == 0),
                                stop=(m == 1),
                                perf_mode=DR,
                            )
                    nc.scalar.activation(
                        out=pt[kt // 2][:, :, kt % 2, :],
                        in_=sc,
                        func=AF.Exp,
                        bias=mbcols[:, kt : kt + 1],
                        scale=SCALE,
                    )

            # ---------------- ctx + residual + layernorm ----------------
            with (
                tc.tile_pool(name="cx", bufs=4, space="PSUM") as cxp,
                tc.tile_pool(name="wk", bufs=4) as work,
            ):
                for qi in range(16):
                    qc, st = qi // 4, qi % 4
                    cs = cxp.tile([128, 2, 512], F32, tag="cs")
                    for jp in range(JP):
                        lhs = pt[jp][:, qc, :, st * 128 : (st + 1) * 128]
                        nc.tensor.matmul(
                            cs[:, 0, 0:256],
                            lhs,
                            v8[jp][:, :, 0:256],
                            start=(jp == 0),
                            stop=(jp == JP - 1),
                            perf_mode=DR,
                        )
                        nc.tensor.matmul(
                            cs[:, 1, 0:257],
                            lhs,
                            v8[jp][:, :, 256 : E + 1],
                            start=(jp == 0),
                            stop=(jp == JP - 1),
                            perf_mode=DR,
                        )
                    recip = work.tile([128, 1], F32, tag="recip")
                    nc.vector.reciprocal(recip, cs[:, 1, 256:257])
                    h = work.tile([128, E], F32, tag="h")
                    nc.vector.scalar_tensor_tensor(
                        out=h[:, 0:256],
                        in0=cs[:, 0, 0:256],
                        scalar=recip,
                        in1=xqst[qc][:, st, 0:256],
                        op0=OP.mult,
                        op1=OP.add,
                    )
                    nc.vector.scalar_tensor_tensor(
                        out=h[:, 256:512],
                        in0=cs[:, 1, 0:256],
                        scalar=recip,
                        in1=xqst[qc][:, st, 256:512],
                        op0=OP.mult,
                        op1=OP.add,
                    )
                    st6 = work.tile([128, 6], F32, tag="st6")
                    nc.vector.bn_stats(out=st6, in_=h)
                    mv = work.tile([128, 2], F32, tag="mv")
                    nc.vector.bn_aggr(out=mv, in_=st6)
                    std = work.tile([128, 1], F32, tag="std")
                    nc.scalar.activation(
                        out=std, in_=mv[:, 1:2], func=AF.Sqrt, bias=eps_t
                    )
                    rstd = work.tile([128, 1], F32, tag="rstd")
                    nc.vector.reciprocal(rstd, std)
                    nmu = work.tile([128, 1], F32, tag="nmu")
                    nc.vector.tensor_scalar(
                        out=nmu,
                        in0=mv[:, 0:1],
                        scalar1=rstd,
                        scalar2=-1.0,
                        op0=OP.mult,
                        op1=OP.mult,
                    )
                    o_t = work.tile([128, E], F32, tag="ot")
                    nc.scalar.activation(
                        out=o_t, in_=h, func=AF.Identity, bias=nmu, scale=rstd
                    )
                    if apply_gb:
                        nc.vector.tensor_mul(o_t, o_t, ga_bc)
                        nc.vector.tensor_add(o_t, o_t, be_bc)
                    nc.gpsimd.dma_start(
                        out=out[qi * 128 : (qi + 1) * 128, :], in_=o_t
                    )
    return nc


# test-harness knobs (the grading harness leaves these at defaults)
TRACE = False
LAST_RESULTS = None


def _ensure_axon_jax():
    """The Bass SPMD run goes through jax/PJRT on the axon platform. If the
    caller pinned jax to cpu (e.g. to run a reference model), unpin it and
    drop any initialized cpu-only backends."""
    import os

    import jax

    try:
        devs = jax.devices()
    except Exception:
        devs = []
    if any(d.platform not in ("cpu",) for d in devs):
        return
    os.environ.pop("JAX_PLATFORMS", None)
    try:
        jax.config.update("jax_platforms", None)
    except Exception:
        pass
    try:
        jax.clear_backends()
    except Exception:
        try:
            jax.extend.backend.clear_backends()
        except Exception:
            pass


def kernel(x, mask, Wq, bq, Wk, bk, Wv, bv, gamma, beta):
    global LAST_RESULTS
    _ensure_axon_jax()
    from concourse.bass_utils import run_bass_kernel_spmd

    x = np.ascontiguousarray(np.asarray(x, dtype=np.float32))
    mask = np.asarray(np.asarray(mask) != 0)
    # Masked keys get softmax weight exactly 0 (exp underflow), so attention
    # only needs the unmasked keys: pack them per core half, padded to a
    # 128 multiple (even tile count); pad slots get the -1e4 bias -> exp==0.
    counts = [
        int(mask[b, h * SQ : (h + 1) * SQ].sum()) for b in range(4) for h in range(2)
    ]
    nkt = max(2, -(-max(counts) // 128))
    nkt += nkt % 2
    S_OWN = nkt * 128
    common = {
        "WqT": np.ascontiguousarray(np.asarray(Wq, dtype=np.float32).T),
        "WkT": np.ascontiguousarray(np.asarray(Wk, dtype=np.float32).T),
        "WvT": np.ascontiguousarray(np.asarray(Wv, dtype=np.float32).T),
        "bq": np.ascontiguousarray(bq, dtype=np.float32),
        "bk": np.ascontiguousarray(bk, dtype=np.float32),
        "bv": np.ascontiguousarray(bv, dtype=np.float32),
        "gamma": np.ascontiguousarray(gamma, dtype=np.float32),
        "beta": np.ascontiguousarray(beta, dtype=np.float32),
        "ident": np.eye(128, dtype=np.float32),
    }

    def packed_kv(b, h):
        rows = x[b, h * SQ : (h + 1) * SQ]
        sel = rows[mask[b, h * SQ : (h + 1) * SQ]]
        xkv = np.zeros((S_OWN, E), dtype=np.float32)
        xkv[: len(sel)] = sel
        mb = np.full(S_OWN, MASK_NEG + SHIFT, dtype=np.float32)
        mb[: len(sel)] = SHIFT
        return xkv, mb

    in_maps = []
    for c in range(8):
        b, h = c // 2, c % 2
        xkv_own, mb_own = packed_kv(b, h)
        _, mb_sib = packed_kv(b, 1 - h)
        # key order inside the kernel is [own packed | sibling packed]
        in_maps.append(
            {
                "xq": np.ascontiguousarray(x[b, h * SQ : (h + 1) * SQ]),
                "xkv": xkv_own,
                "maskbias": np.concatenate([mb_own, mb_sib]),
                **common,
            }
        )
    apply_gb = not (
        np.all(np.asarray(gamma) == 1.0) and np.all(np.asarray(beta) == 0.0)
    )
    nc = build_nc(nkt, apply_gb)
    nc.compile()
    res = run_bass_kernel_spmd(nc, in_maps, core_ids=list(range(8)), trace=TRACE)
    LAST_RESULTS = res
    full = np.empty((4, S, E), dtype=np.float32)
    for c in range(8):
        b, h = c // 2, c % 2
        full[b, h * SQ : (h + 1) * SQ] = res.results[c]["out"]
    return full
